# revision 10
# baseline (speedup 1.0000x reference)
"""AdaPT quantized linear (int8-exact via bf16 matmul) on 8 TRN2 NeuronCores.

Reference computes:
    qx = clip(round(x * 127/amax_x), -127, 127)        [N, K] int8
    qw = clip(round(w * 127/amax_w), -127, 127)        [M, K] int8
    out = (qx @ qw.T) / ((127/amax_x)*(127/amax_w)) + bias

Strategy: data-parallel over the 8192-token dim (1024 tokens/core), full
weight on every core, no collectives.  All int8 values are exactly
representable in bf16 (8-bit significand), the PE multiplies bf16 exactly
(products < 2^14) and accumulates in fp32 (sums << 2^24), so the bf16
matmul reproduces the int8 systolic GEMM bit-exactly.  Rounding uses the
+1.5*2^23 magic-constant trick which matches round-half-to-even.

Per-core device kernel (weight-stationary):
  - quantize x.T shard once into resident SBUF bf16 [128, 32, 1024]
  - per m-tile (128 rows of W): DMA w.T block, quantize, 64 accumulating
    matmuls (32 k-tiles x 2 token chunks of 512), dequant+bias on ScalarE
    straight out of PSUM, DMA out.

Output is produced transposed ([M, tokens/core] per core) so the bias can
ride the ScalarE per-partition bias port; host transposes back.
"""

import sys

import numpy as np

sys.path.insert(0, "/opt/trn_rl_repo")

N, K, M = 8192, 4096, 4096
N_CORES = 8
TPC = N // N_CORES  # tokens per core
P = 128
KT = K // P   # 32 k-tiles
MT = M // P   # 32 m-tiles
TF = 512      # matmul moving free dim (one PSUM bank of fp32)
NTF = TPC // TF
XCH = 8       # resident xq is split into chunks for finer scheduling deps
KPC = KT // XCH
MAGIC = float(1.5 * 2**23)  # 12582912.0; +MAGIC then -MAGIC rounds to int (RNE)
MAXV = 127.0


def build(s_x: float, s_w: float, inv_s: float):
    import concourse.mybir as mybir
    import concourse.tile as tile
    from concourse import bacc

    dt = mybir.dt
    AF = mybir.ActivationFunctionType
    OP = mybir.AluOpType

    nc = bacc.Bacc("TRN2", target_bir_lowering=False, debug=False,
                   num_devices=N_CORES)

    # activation() lowers non-Copy bias to a const AP; register the magic
    # rounding constants the same way Bass registers 0.0/1.0 at init.
    for v in (MAGIC, -MAGIC):
        t = nc.alloc_sbuf_tensor(f"const-float32-{v}", [128, 1], dt.float32)
        nc.gpsimd.memset(t.ap(), v)
        nc.const_aps.aps[(dt.float32, v)] = t.ap()
    nc.all_engine_barrier()

    xt = nc.declare_dram_parameter("xt", [K, TPC], dt.float32, isOutput=False)
    wt = nc.declare_dram_parameter("wt", [MT, K, P], dt.float32, isOutput=False)
    bias = nc.declare_dram_parameter("bias", [M], dt.float32, isOutput=False)
    out = nc.declare_dram_parameter("out", [M, TPC], dt.float32, isOutput=True)

    with tile.TileContext(nc) as tc:
        with (
            tc.tile_pool(name="xq", bufs=1) as xq_pool,
            tc.tile_pool(name="xs", bufs=4) as xs_pool,
            tc.tile_pool(name="ws", bufs=3) as ws_pool,
            tc.tile_pool(name="wq", bufs=4) as wq_pool,
            tc.tile_pool(name="cst", bufs=1) as cst_pool,
            tc.tile_pool(name="outp", bufs=5) as out_pool,
            tc.tile_pool(name="ps", bufs=3, space="PSUM") as psum_pool,
            tc.tile_pool(name="junk", bufs=1) as junk_pool,
            tc.tile_pool(name="psjunk", bufs=1, space="PSUM") as psjunk_pool,
        ):
            bias_sb = cst_pool.tile([P, MT], dt.float32, name="bias_sb")
            nc.sync.dma_start(bias_sb[:], bias[:].rearrange("(o p) -> p o", p=P))

            # PE warmup: junk matmuls keep HAM hot while the first x/w
            # chunks quantize; they retire before any real matmul is ready.
            junk_sb = junk_pool.tile([P, TF], dt.bfloat16, name="junk_sb")
            junk_ps = psjunk_pool.tile([P, TF], dt.float32, name="junk_ps")
            nc.vector.memset(junk_sb[:], 1.0)
            def junk_mms(n):
                for _ in range(n):
                    nc.tensor.matmul(junk_ps[:], junk_sb[:, :P], junk_sb[:],
                                     start=True, stop=True)

            junk_mms(40)

            def quantize(dst, src, scale, p1, p3):
                # dst (bf16) = clip(round(src * scale), -127, 127), exactly
                # matching jnp.round (half-to-even) + jnp.clip.  The clip
                # (min/max pair) only runs on DVE; the affine passes run on
                # the engine given by p1/p3 ("act" or "dve") so chains can
                # be balanced across ScalarE and VectorE.
                if p1 == "act":
                    nc.scalar.activation(src, src, AF.Identity,
                                         bias=MAGIC, scale=scale)
                else:
                    nc.vector.tensor_scalar(src, src, scale, MAGIC,
                                            OP.mult, OP.add)
                nc.vector.tensor_scalar(src, src, MAGIC + MAXV, MAGIC - MAXV,
                                        OP.min, OP.max)
                if p3 == "act":
                    nc.scalar.activation(dst, src, AF.Identity, bias=-MAGIC)
                else:
                    nc.vector.tensor_scalar(dst, src, MAGIC, None,
                                            OP.subtract)

            xq_tiles = [
                xq_pool.tile([P, KPC, TPC], dt.bfloat16, name=f"xq{c}", tag=f"xq{c}")
                for c in range(XCH)
            ]

            def quant_x(kt):
                xs = xs_pool.tile([P, TPC], dt.float32, name="xs")
                nc.sync.dma_start(xs[:], xt[kt * P:(kt + 1) * P, :])
                if kt % 2 == 0:
                    quantize(xq_tiles[kt // KPC][:, kt % KPC, :], xs[:], s_x,
                             p1="act", p3="act")
                else:
                    quantize(xq_tiles[kt // KPC][:, kt % KPC, :], xs[:], s_x,
                             p1="dve", p3="dve")

            def prep_w(mt):
                ws = ws_pool.tile([P, KT, P], dt.float32, name="ws")
                for q in range(XCH):
                    nc.sync.dma_start(
                        ws[:, q * KPC:(q + 1) * KPC, :],
                        wt[mt, q * KPC * P:(q + 1) * KPC * P, :]
                        .rearrange("(o p) f -> p o f", p=P),
                    )
                wq = wq_pool.tile([P, KT, P], dt.bfloat16, name="wq")
                quantize(wq[:], ws[:], s_w, p1="act", p3="dve")
                return wq

            def alloc_ps():
                return [psum_pool.tile([P, TF], dt.float32, name=f"ps{i}")
                        for i in range(NTF)]

            def mm(pss, wq, kt, start, stop):
                for tf in range(NTF):
                    nc.tensor.matmul(
                        pss[tf][:],
                        wq[:, kt, :],
                        xq_tiles[kt // KPC][:, kt % KPC,
                                           tf * TF:(tf + 1) * TF],
                        start=start, stop=stop,
                    )

            def store(mt, pss):
                outt = out_pool.tile([P, TPC], dt.float32, name="outt")
                for tf in range(NTF):
                    nc.scalar.activation(
                        outt[:, tf * TF:(tf + 1) * TF], pss[tf][:],
                        AF.Identity, bias=bias_sb[:, mt:mt + 1], scale=inv_s,
                    )
                    nc.sync.dma_start(
                        out[mt * P:(mt + 1) * P, tf * TF:(tf + 1) * TF],
                        outt[:, tf * TF:(tf + 1) * TF])

            # Fused prologue: quantize x chunk-by-chunk, staggered with the
            # first PRO m-tiles' weight prep; after each chunk, run the
            # matmuls that are newly enabled (psum accumulation k-order is
            # free, each m-tile still sees chunks in order).  PE gets work
            # as soon as (wq0, x-chunk0) exist.  3 m-tiles x 2 psum banks
            # + 1 junk bank fit in PSUM.
            PRO = min(3, MT)
            wqs = {}
            pro_ps = {}
            wqs[0] = prep_w(0)
            pro_ps[0] = alloc_ps()
            for c in range(XCH):
                for k in range(KPC):
                    quant_x(c * KPC + k)
                if c + 1 < PRO:
                    wqs[c + 1] = prep_w(c + 1)
                    pro_ps[c + 1] = alloc_ps()
                # mt may use chunk c' = c - mt (staggered start)
                for mt in range(PRO):
                    cc = c - mt
                    if 0 <= cc < XCH:
                        for k in range(KPC):
                            kt = cc * KPC + k
                            mm(pro_ps[mt], wqs[mt], kt,
                               start=(kt == 0), stop=(kt == KT - 1))
                if c < 2:
                    junk_mms(10)
            # drain the staggered tail: mt finishes chunks XCH-mt .. XCH-1
            for mt in range(PRO):
                for cc in range(XCH - mt, XCH):
                    for k in range(KPC):
                        kt = cc * KPC + k
                        mm(pro_ps[mt], wqs[mt], kt,
                           start=(kt == 0), stop=(kt == KT - 1))
                store(mt, pro_ps[mt])

            # Steady-state m-loop, software-pipelined: emit m-tile mt+1's
            # DMA+quant before mt's matmuls so the prefetch always leads.
            pending = {}
            for mt in range(PRO, min(PRO + 2, MT)):
                pending[mt] = prep_w(mt)
            for mt in range(PRO, MT):
                wq = pending.pop(mt)
                if mt + 2 < MT:
                    pending[mt + 2] = prep_w(mt + 2)
                pss = alloc_ps()
                for kt in range(KT):
                    mm(pss, wq, kt, start=(kt == 0), stop=(kt == KT - 1))
                store(mt, pss)

    nc.compile()
    return nc


def _prep(x, weight, bias, amax_x, amax_w):
    ax = np.float32(np.asarray(amax_x, dtype=np.float32).reshape(-1)[0])
    aw = np.float32(np.asarray(amax_w, dtype=np.float32).reshape(-1)[0])
    s_x = np.float32(127.0) / ax
    s_w = np.float32(127.0) / aw
    inv_s = np.float32(1.0) / (s_x * s_w)

    x = np.asarray(x, dtype=np.float32)
    weight = np.asarray(weight, dtype=np.float32)
    bias = np.asarray(bias, dtype=np.float32)

    xT = np.ascontiguousarray(x.T)  # [K, N]
    # [MT, K, 128]: per m-tile a contiguous k-major block of W^T
    wt3 = np.ascontiguousarray(weight.reshape(MT, P, K).transpose(0, 2, 1))
    in_maps = [
        {
            "xt": np.ascontiguousarray(xT[:, c * TPC:(c + 1) * TPC]),
            "wt": wt3,
            "bias": bias,
        }
        for c in range(N_CORES)
    ]
    return float(s_x), float(s_w), float(inv_s), in_maps


def run(x, weight, bias, amax_x, amax_w, trace: bool = False):
    from concourse.bass_utils import run_bass_kernel_spmd

    s_x, s_w, inv_s, in_maps = _prep(x, weight, bias, amax_x, amax_w)
    nc = build(s_x, s_w, inv_s)
    res = run_bass_kernel_spmd(nc, in_maps, core_ids=list(range(N_CORES)),
                               trace=trace)
    shards = [res.results[c]["out"] for c in range(N_CORES)]
    full = np.concatenate([s.T for s in shards], axis=0).astype(np.float32)
    return full, res


def kernel(x, weight, bias, amax_x, amax_w):
    full, _ = run(x, weight, bias, amax_x, amax_w, trace=False)
    return full


# revision 11
# speedup vs baseline: 1.0690x; 1.0690x over previous
"""AdaPT quantized linear (int8-exact via bf16 matmul) on 8 TRN2 NeuronCores.

Reference computes:
    qx = clip(round(x * 127/amax_x), -127, 127)        [N, K] int8
    qw = clip(round(w * 127/amax_w), -127, 127)        [M, K] int8
    out = (qx @ qw.T) / ((127/amax_x)*(127/amax_w)) + bias

Strategy: data-parallel over the 8192-token dim (1024 tokens/core), full
weight on every core, no collectives.  All int8 values are exactly
representable in bf16 (8-bit significand), the PE multiplies bf16 exactly
(products < 2^14) and accumulates in fp32 (sums << 2^24), so the bf16
matmul reproduces the int8 systolic GEMM bit-exactly.  Rounding uses the
+1.5*2^23 magic-constant trick which matches round-half-to-even.

Per-core device kernel (weight-stationary):
  - quantize x.T shard once into resident SBUF bf16 [128, 32, 1024]
  - per m-tile (128 rows of W): DMA w.T block, quantize, 64 accumulating
    matmuls (32 k-tiles x 2 token chunks of 512), dequant+bias on ScalarE
    straight out of PSUM, DMA out.

Output is produced transposed ([M, tokens/core] per core) so the bias can
ride the ScalarE per-partition bias port; host transposes back.
"""

import sys

import numpy as np

sys.path.insert(0, "/opt/trn_rl_repo")

N, K, M = 8192, 4096, 4096
N_CORES = 8
TPC = N // N_CORES  # tokens per core
P = 128
KT = K // P   # 32 k-tiles
MT = M // P   # 32 m-tiles
TF = 512      # matmul moving free dim (one PSUM bank of fp32)
NTF = TPC // TF
XCH = 4       # resident xq is split into chunks for finer scheduling deps
KPC = KT // XCH
MAGIC = float(1.5 * 2**23)  # 12582912.0; +MAGIC then -MAGIC rounds to int (RNE)
MAXV = 127.0


def build(s_x: float, s_w: float, inv_s: float):
    import concourse.mybir as mybir
    import concourse.tile as tile
    from concourse import bacc

    dt = mybir.dt
    AF = mybir.ActivationFunctionType
    OP = mybir.AluOpType

    nc = bacc.Bacc("TRN2", target_bir_lowering=False, debug=False,
                   num_devices=N_CORES)

    # activation() lowers non-Copy bias to a const AP; register the magic
    # rounding constants the same way Bass registers 0.0/1.0 at init.
    for v in (MAGIC, -MAGIC):
        t = nc.alloc_sbuf_tensor(f"const-float32-{v}", [128, 1], dt.float32)
        nc.gpsimd.memset(t.ap(), v)
        nc.const_aps.aps[(dt.float32, v)] = t.ap()
    nc.all_engine_barrier()

    xt = nc.declare_dram_parameter("xt", [K, TPC], dt.float32, isOutput=False)
    wt = nc.declare_dram_parameter("wt", [MT, K, P], dt.float32, isOutput=False)
    bias = nc.declare_dram_parameter("bias", [M], dt.float32, isOutput=False)
    out = nc.declare_dram_parameter("out", [M, TPC], dt.float32, isOutput=True)

    with tile.TileContext(nc) as tc:
        with (
            tc.tile_pool(name="xq", bufs=1) as xq_pool,
            tc.tile_pool(name="xs", bufs=4) as xs_pool,
            tc.tile_pool(name="ws", bufs=3) as ws_pool,
            tc.tile_pool(name="wq", bufs=4) as wq_pool,
            tc.tile_pool(name="cst", bufs=1) as cst_pool,
            tc.tile_pool(name="outp", bufs=5) as out_pool,
            tc.tile_pool(name="ps", bufs=3, space="PSUM") as psum_pool,
            tc.tile_pool(name="junk", bufs=1) as junk_pool,
            tc.tile_pool(name="psjunk", bufs=1, space="PSUM") as psjunk_pool,
        ):
            bias_sb = cst_pool.tile([P, MT], dt.float32, name="bias_sb")
            nc.sync.dma_start(bias_sb[:], bias[:].rearrange("(o p) -> p o", p=P))

            # PE warmup: junk matmuls keep HAM hot while the first x/w
            # chunks quantize; they retire before any real matmul is ready.
            junk_sb = junk_pool.tile([P, TF], dt.bfloat16, name="junk_sb")
            junk_ps = psjunk_pool.tile([P, TF], dt.float32, name="junk_ps")
            nc.vector.memset(junk_sb[:], 1.0)
            def junk_mms(n):
                for _ in range(n):
                    nc.tensor.matmul(junk_ps[:], junk_sb[:, :P], junk_sb[:],
                                     start=True, stop=True)

            junk_mms(40)

            def quantize(dst, src, scale, p1, p3):
                # dst (bf16) = clip(round(src * scale), -127, 127), exactly
                # matching jnp.round (half-to-even) + jnp.clip.  The clip
                # (min/max pair) only runs on DVE; the affine passes run on
                # the engine given by p1/p3 ("act" or "dve") so chains can
                # be balanced across ScalarE and VectorE.
                if p1 == "act":
                    nc.scalar.activation(src, src, AF.Identity,
                                         bias=MAGIC, scale=scale)
                else:
                    nc.vector.tensor_scalar(src, src, scale, MAGIC,
                                            OP.mult, OP.add)
                nc.vector.tensor_scalar(src, src, MAGIC + MAXV, MAGIC - MAXV,
                                        OP.min, OP.max)
                if p3 == "act":
                    nc.scalar.activation(dst, src, AF.Identity, bias=-MAGIC)
                else:
                    nc.vector.tensor_scalar(dst, src, MAGIC, None,
                                            OP.subtract)

            xq_tiles = [
                xq_pool.tile([P, KPC, TPC], dt.bfloat16, name=f"xq{c}", tag=f"xq{c}")
                for c in range(XCH)
            ]

            def quant_x(kt):
                xs = xs_pool.tile([P, TPC], dt.float32, name="xs")
                nc.sync.dma_start(xs[:], xt[kt * P:(kt + 1) * P, :])
                if kt % 2 == 0:
                    quantize(xq_tiles[kt // KPC][:, kt % KPC, :], xs[:], s_x,
                             p1="act", p3="act")
                else:
                    quantize(xq_tiles[kt // KPC][:, kt % KPC, :], xs[:], s_x,
                             p1="dve", p3="dve")

            WCH = 4
            WKC = KT // WCH

            def prep_w(mt):
                # 4 independently-pipelined sub-chunks: DMA -> quant per
                # chunk, each landing in its own wq tile so the m-tile's
                # first matmuls only wait on chunk 0.
                wqs = []
                for q in range(WCH):
                    ws = ws_pool.tile([P, WKC, P], dt.float32, name="ws",
                                      tag=f"ws{q}")
                    nc.sync.dma_start(
                        ws[:],
                        wt[mt, q * WKC * P:(q + 1) * WKC * P, :]
                        .rearrange("(o p) f -> p o f", p=P),
                    )
                    wq = wq_pool.tile([P, WKC, P], dt.bfloat16, name="wq",
                                      tag=f"wq{q}")
                    quantize(wq[:], ws[:], s_w, p1="act", p3="dve")
                    wqs.append(wq)
                return wqs

            def alloc_ps():
                return [psum_pool.tile([P, TF], dt.float32, name=f"ps{i}")
                        for i in range(NTF)]

            def mm(pss, wqs, kt, start, stop):
                for tf in range(NTF):
                    nc.tensor.matmul(
                        pss[tf][:],
                        wqs[kt // WKC][:, kt % WKC, :],
                        xq_tiles[kt // KPC][:, kt % KPC,
                                           tf * TF:(tf + 1) * TF],
                        start=start, stop=stop,
                    )

            def store(mt, pss):
                outt = out_pool.tile([P, TPC], dt.float32, name="outt")
                for tf in range(NTF):
                    nc.scalar.activation(
                        outt[:, tf * TF:(tf + 1) * TF], pss[tf][:],
                        AF.Identity, bias=bias_sb[:, mt:mt + 1], scale=inv_s,
                    )
                    nc.sync.dma_start(
                        out[mt * P:(mt + 1) * P, tf * TF:(tf + 1) * TF],
                        outt[:, tf * TF:(tf + 1) * TF])

            # Fused prologue: quantize x chunk-by-chunk, staggered with the
            # first PRO m-tiles' weight prep; after each chunk, run the
            # matmuls that are newly enabled (psum accumulation k-order is
            # free, each m-tile still sees chunks in order).  PE gets work
            # as soon as (wq0, x-chunk0) exist.  3 m-tiles x 2 psum banks
            # + 1 junk bank fit in PSUM.
            PRO = min(3, MT)
            wqs = {}
            pro_ps = {}
            wqs[0] = prep_w(0)
            pro_ps[0] = alloc_ps()
            for c in range(XCH):
                for k in range(KPC):
                    quant_x(c * KPC + k)
                if c + 1 < PRO:
                    wqs[c + 1] = prep_w(c + 1)
                    pro_ps[c + 1] = alloc_ps()
                # mt may use chunk c' = c - mt (staggered start)
                for mt in range(PRO):
                    cc = c - mt
                    if 0 <= cc < XCH:
                        for k in range(KPC):
                            kt = cc * KPC + k
                            mm(pro_ps[mt], wqs[mt], kt,
                               start=(kt == 0), stop=(kt == KT - 1))
            # drain the staggered tail: mt finishes chunks XCH-mt .. XCH-1
            for mt in range(PRO):
                for cc in range(XCH - mt, XCH):
                    for k in range(KPC):
                        kt = cc * KPC + k
                        mm(pro_ps[mt], wqs[mt], kt,
                           start=(kt == 0), stop=(kt == KT - 1))
                store(mt, pro_ps[mt])

            # Steady-state m-loop, software-pipelined: emit m-tile mt+1's
            # DMA+quant before mt's matmuls so the prefetch always leads.
            pending = {}
            for mt in range(PRO, min(PRO + 2, MT)):
                pending[mt] = prep_w(mt)
            for mt in range(PRO, MT):
                wq = pending.pop(mt)
                if mt + 2 < MT:
                    pending[mt + 2] = prep_w(mt + 2)
                pss = alloc_ps()
                for kt in range(KT):
                    mm(pss, wq, kt, start=(kt == 0), stop=(kt == KT - 1))
                store(mt, pss)

    nc.compile()
    return nc


def _prep(x, weight, bias, amax_x, amax_w):
    ax = np.float32(np.asarray(amax_x, dtype=np.float32).reshape(-1)[0])
    aw = np.float32(np.asarray(amax_w, dtype=np.float32).reshape(-1)[0])
    s_x = np.float32(127.0) / ax
    s_w = np.float32(127.0) / aw
    inv_s = np.float32(1.0) / (s_x * s_w)

    x = np.asarray(x, dtype=np.float32)
    weight = np.asarray(weight, dtype=np.float32)
    bias = np.asarray(bias, dtype=np.float32)

    xT = np.ascontiguousarray(x.T)  # [K, N]
    # [MT, K, 128]: per m-tile a contiguous k-major block of W^T
    wt3 = np.ascontiguousarray(weight.reshape(MT, P, K).transpose(0, 2, 1))
    in_maps = [
        {
            "xt": np.ascontiguousarray(xT[:, c * TPC:(c + 1) * TPC]),
            "wt": wt3,
            "bias": bias,
        }
        for c in range(N_CORES)
    ]
    return float(s_x), float(s_w), float(inv_s), in_maps


def run(x, weight, bias, amax_x, amax_w, trace: bool = False):
    from concourse.bass_utils import run_bass_kernel_spmd

    s_x, s_w, inv_s, in_maps = _prep(x, weight, bias, amax_x, amax_w)
    nc = build(s_x, s_w, inv_s)
    res = run_bass_kernel_spmd(nc, in_maps, core_ids=list(range(N_CORES)),
                               trace=trace)
    shards = [res.results[c]["out"] for c in range(N_CORES)]
    full = np.concatenate([s.T for s in shards], axis=0).astype(np.float32)
    return full, res


def kernel(x, weight, bias, amax_x, amax_w):
    full, _ = run(x, weight, bias, amax_x, amax_w, trace=False)
    return full


# revision 12
# speedup vs baseline: 1.0933x; 1.0227x over previous
"""AdaPT quantized linear (int8-exact via bf16 matmul) on 8 TRN2 NeuronCores.

Reference computes:
    qx = clip(round(x * 127/amax_x), -127, 127)        [N, K] int8
    qw = clip(round(w * 127/amax_w), -127, 127)        [M, K] int8
    out = (qx @ qw.T) / ((127/amax_x)*(127/amax_w)) + bias

Strategy: data-parallel over the 8192-token dim (1024 tokens/core), full
weight on every core, no collectives.  All int8 values are exactly
representable in bf16 (8-bit significand), the PE multiplies bf16 exactly
(products < 2^14) and accumulates in fp32 (sums << 2^24), so the bf16
matmul reproduces the int8 systolic GEMM bit-exactly.  Rounding uses the
+1.5*2^23 magic-constant trick which matches round-half-to-even.

Per-core device kernel (weight-stationary):
  - quantize x.T shard once into resident SBUF bf16 [128, 32, 1024]
  - per m-tile (128 rows of W): DMA w.T block, quantize, 64 accumulating
    matmuls (32 k-tiles x 2 token chunks of 512), dequant+bias on ScalarE
    straight out of PSUM, DMA out.

Output is produced transposed ([M, tokens/core] per core) so the bias can
ride the ScalarE per-partition bias port; host transposes back.
"""

import sys

import numpy as np

sys.path.insert(0, "/opt/trn_rl_repo")

N, K, M = 8192, 4096, 4096
N_CORES = 8
TPC = N // N_CORES  # tokens per core
P = 128
KT = K // P   # 32 k-tiles
MT = M // P   # 32 m-tiles
TF = 512      # matmul moving free dim (one PSUM bank of fp32)
NTF = TPC // TF
XCH = 4       # resident xq is split into chunks for finer scheduling deps
KPC = KT // XCH
MAGIC = float(1.5 * 2**23)  # 12582912.0; +MAGIC then -MAGIC rounds to int (RNE)
MAXV = 127.0


def build(s_x: float, s_w: float, inv_s: float):
    import concourse.mybir as mybir
    import concourse.tile as tile
    from concourse import bacc

    dt = mybir.dt
    AF = mybir.ActivationFunctionType
    OP = mybir.AluOpType

    nc = bacc.Bacc("TRN2", target_bir_lowering=False, debug=False,
                   num_devices=N_CORES)

    # activation() lowers non-Copy bias to a const AP; register the magic
    # rounding constants the same way Bass registers 0.0/1.0 at init.
    for v in (MAGIC, -MAGIC):
        t = nc.alloc_sbuf_tensor(f"const-float32-{v}", [128, 1], dt.float32)
        nc.gpsimd.memset(t.ap(), v)
        nc.const_aps.aps[(dt.float32, v)] = t.ap()
    nc.all_engine_barrier()

    xt = nc.declare_dram_parameter("xt", [K, TPC], dt.float32, isOutput=False)
    wt = nc.declare_dram_parameter("wt", [MT, K, P], dt.float32, isOutput=False)
    bias = nc.declare_dram_parameter("bias", [M], dt.float32, isOutput=False)
    out = nc.declare_dram_parameter("out", [M, TPC], dt.float32, isOutput=True)

    with tile.TileContext(nc) as tc:
        with (
            tc.tile_pool(name="xq", bufs=1) as xq_pool,
            tc.tile_pool(name="xs", bufs=6) as xs_pool,
            tc.tile_pool(name="ws", bufs=3) as ws_pool,
            tc.tile_pool(name="wq", bufs=4) as wq_pool,
            tc.tile_pool(name="cst", bufs=1) as cst_pool,
            tc.tile_pool(name="outp", bufs=5) as out_pool,
            tc.tile_pool(name="ps", bufs=4, space="PSUM") as psum_pool,
            tc.tile_pool(name="junk", bufs=1) as junk_pool,
        ):
            bias_sb = cst_pool.tile([P, MT], dt.float32, name="bias_sb")
            nc.sync.dma_start(bias_sb[:], bias[:].rearrange("(o p) -> p o", p=P))

            # PE warmup: junk matmuls keep HAM hot while the first x/w
            # chunks quantize; they retire before any real matmul is ready.
            junk_sb = junk_pool.tile([P, TF], dt.bfloat16, name="junk_sb")
            nc.vector.memset(junk_sb[:], 1.0)

            def junk_mms(n, tgt):
                for _ in range(n):
                    nc.tensor.matmul(tgt[:], junk_sb[:, :P], junk_sb[:],
                                     start=True, stop=True)

            def quantize(dst, src, scale, p1, p3):
                # dst (bf16) = clip(round(src * scale), -127, 127), exactly
                # matching jnp.round (half-to-even) + jnp.clip.  The clip
                # (min/max pair) only runs on DVE; the affine passes run on
                # the engine given by p1/p3 ("act" or "dve") so chains can
                # be balanced across ScalarE and VectorE.
                if p1 == "act":
                    nc.scalar.activation(src, src, AF.Identity,
                                         bias=MAGIC, scale=scale)
                else:
                    nc.vector.tensor_scalar(src, src, scale, MAGIC,
                                            OP.mult, OP.add)
                nc.vector.tensor_scalar(src, src, MAGIC + MAXV, MAGIC - MAXV,
                                        OP.min, OP.max)
                if p3 == "act":
                    nc.scalar.activation(dst, src, AF.Identity, bias=-MAGIC)
                else:
                    nc.vector.tensor_scalar(dst, src, MAGIC, None,
                                            OP.subtract)

            xq_tiles = [
                xq_pool.tile([P, KPC, TPC], dt.bfloat16, name=f"xq{c}", tag=f"xq{c}")
                for c in range(XCH)
            ]

            def quant_x(kt):
                xs = xs_pool.tile([P, TPC], dt.float32, name="xs")
                nc.sync.dma_start(xs[:], xt[kt * P:(kt + 1) * P, :])
                dst = xq_tiles[kt // KPC][:, kt % KPC, :]
                if kt % 2 == 0:
                    nc.scalar.activation(xs[:], xs[:], AF.Identity,
                                         bias=MAGIC, scale=s_x)
                    nc.vector.tensor_scalar(xs[:], xs[:], MAGIC + MAXV,
                                            MAGIC - MAXV, OP.min, OP.max)
                    nc.scalar.activation(dst, xs[:], AF.Identity, bias=-MAGIC)
                else:
                    nc.vector.tensor_scalar(xs[:], xs[:], s_x, MAGIC,
                                            OP.mult, OP.add)
                    nc.gpsimd.tensor_scalar(xs[:], xs[:], MAGIC + MAXV,
                                            MAGIC - MAXV, OP.min, OP.max)
                    nc.vector.tensor_scalar(dst, xs[:], MAGIC, None,
                                            OP.subtract)

            WCH = 4
            WKC = KT // WCH

            def prep_w(mt):
                # 4 independently-pipelined sub-chunks: DMA -> quant per
                # chunk, each landing in its own wq tile so the m-tile's
                # first matmuls only wait on chunk 0.
                wqs = []
                for q in range(WCH):
                    ws = ws_pool.tile([P, WKC, P], dt.float32, name="ws",
                                      tag=f"ws{q}")
                    nc.sync.dma_start(
                        ws[:],
                        wt[mt, q * WKC * P:(q + 1) * WKC * P, :]
                        .rearrange("(o p) f -> p o f", p=P),
                    )
                    wq = wq_pool.tile([P, WKC, P], dt.bfloat16, name="wq",
                                      tag=f"wq{q}")
                    quantize(wq[:], ws[:], s_w, p1="act", p3="dve")
                    wqs.append(wq)
                return wqs

            def alloc_ps():
                return [psum_pool.tile([P, TF], dt.float32, name=f"ps{i}")
                        for i in range(NTF)]

            def mm(pss, wqs, kt, start, stop):
                for tf in range(NTF):
                    nc.tensor.matmul(
                        pss[tf][:],
                        wqs[kt // WKC][:, kt % WKC, :],
                        xq_tiles[kt // KPC][:, kt % KPC,
                                           tf * TF:(tf + 1) * TF],
                        start=start, stop=stop,
                    )

            def store(mt, pss):
                outt = out_pool.tile([P, TPC], dt.float32, name="outt")
                for tf in range(NTF):
                    nc.scalar.activation(
                        outt[:, tf * TF:(tf + 1) * TF], pss[tf][:],
                        AF.Identity, bias=bias_sb[:, mt:mt + 1], scale=inv_s,
                    )
                    nc.sync.dma_start(
                        out[mt * P:(mt + 1) * P, tf * TF:(tf + 1) * TF],
                        outt[:, tf * TF:(tf + 1) * TF])

            # Fused prologue: quantize x chunk-by-chunk, staggered with the
            # first PRO m-tiles' weight prep; after each chunk, run the
            # matmuls that are newly enabled (psum accumulation k-order is
            # free, each m-tile still sees chunks in order).  PE gets work
            # as soon as (wq0, x-chunk0) exist.  3 m-tiles x 2 psum banks
            # + 1 junk bank fit in PSUM.
            PRO = min(4, MT)
            wqs = {}
            pro_ps = {}
            pro_ps[PRO - 1] = alloc_ps()
            junk_mms(40, pro_ps[PRO - 1][0])
            wqs[0] = prep_w(0)
            pro_ps[0] = alloc_ps()
            for c in range(XCH):
                for k in range(KPC):
                    quant_x(c * KPC + k)
                if c + 1 < PRO:
                    wqs[c + 1] = prep_w(c + 1)
                    if c + 1 not in pro_ps:
                        pro_ps[c + 1] = alloc_ps()
                # mt may use chunk c' = c - mt (staggered start)
                for mt in range(PRO):
                    cc = c - mt
                    if 0 <= cc < XCH:
                        for k in range(KPC):
                            kt = cc * KPC + k
                            mm(pro_ps[mt], wqs[mt], kt,
                               start=(kt == 0), stop=(kt == KT - 1))
            # drain the staggered tail: mt finishes chunks XCH-mt .. XCH-1
            for mt in range(PRO):
                for cc in range(XCH - mt, XCH):
                    for k in range(KPC):
                        kt = cc * KPC + k
                        mm(pro_ps[mt], wqs[mt], kt,
                           start=(kt == 0), stop=(kt == KT - 1))
                store(mt, pro_ps[mt])

            # Steady-state m-loop, software-pipelined: emit m-tile mt+1's
            # DMA+quant before mt's matmuls so the prefetch always leads.
            pending = {}
            for mt in range(PRO, min(PRO + 2, MT)):
                pending[mt] = prep_w(mt)
            for mt in range(PRO, MT):
                wq = pending.pop(mt)
                if mt + 2 < MT:
                    pending[mt + 2] = prep_w(mt + 2)
                pss = alloc_ps()
                for kt in range(KT):
                    mm(pss, wq, kt, start=(kt == 0), stop=(kt == KT - 1))
                store(mt, pss)

    nc.compile()
    return nc


def _prep(x, weight, bias, amax_x, amax_w):
    ax = np.float32(np.asarray(amax_x, dtype=np.float32).reshape(-1)[0])
    aw = np.float32(np.asarray(amax_w, dtype=np.float32).reshape(-1)[0])
    s_x = np.float32(127.0) / ax
    s_w = np.float32(127.0) / aw
    inv_s = np.float32(1.0) / (s_x * s_w)

    x = np.asarray(x, dtype=np.float32)
    weight = np.asarray(weight, dtype=np.float32)
    bias = np.asarray(bias, dtype=np.float32)

    xT = np.ascontiguousarray(x.T)  # [K, N]
    # [MT, K, 128]: per m-tile a contiguous k-major block of W^T
    wt3 = np.ascontiguousarray(weight.reshape(MT, P, K).transpose(0, 2, 1))
    in_maps = [
        {
            "xt": np.ascontiguousarray(xT[:, c * TPC:(c + 1) * TPC]),
            "wt": wt3,
            "bias": bias,
        }
        for c in range(N_CORES)
    ]
    return float(s_x), float(s_w), float(inv_s), in_maps


def run(x, weight, bias, amax_x, amax_w, trace: bool = False):
    from concourse.bass_utils import run_bass_kernel_spmd

    s_x, s_w, inv_s, in_maps = _prep(x, weight, bias, amax_x, amax_w)
    nc = build(s_x, s_w, inv_s)
    res = run_bass_kernel_spmd(nc, in_maps, core_ids=list(range(N_CORES)),
                               trace=trace)
    shards = [res.results[c]["out"] for c in range(N_CORES)]
    full = np.concatenate([s.T for s in shards], axis=0).astype(np.float32)
    return full, res


def kernel(x, weight, bias, amax_x, amax_w):
    full, _ = run(x, weight, bias, amax_x, amax_w, trace=False)
    return full
